# revision 6
# baseline (speedup 1.0000x reference)
"""MeanAggregator (GNN mean message passing) Trainium2 kernel.

out[b, :] = mean_s features_table[neigh_idx[b, s], :]   b < 100000, s < 10

Strategy (data-parallel over the batch, 8 cores):

The SWDGE `dma_gather` primitive gathers thousands of 512B rows per
instruction but takes int16 indices (< 32768). Host-side we relabel:
nodes are split into 33 groups of 3072; each group references at most
3072*10 = 30720 unique table rows, so the host packs those rows into a
dedicated 32768-row chunk of a rebuilt table and rewrites the group's
indices as chunk-local int16. Every gather is then int16-addressable
AND in node order -- one gather per (group, core) lands the 10 neighbor
rows of 384 nodes s-major in SBUF, a 4-instruction DVE tree-sum
produces the 384 means, and a contiguous store writes them out. Rotating
the gathers across the 4 SWDGE queues lifts the DMA ceiling from
~52 GB/s (one queue's engines) to ~208 GB/s per core.

The table is pre-scaled by 1/10 on the host so the tree-sum directly
yields the mean; the per-core outputs are permuted back to the original
node order on the host.
"""

import numpy as np

P = 128            # SBUF partitions
D = 128            # feature dim
S = 10             # neighbors per node
N_NODES = 1_000_000
BATCH = 100_000
N_CORES = 8

GROUP = 3072               # nodes per group (unique refs <= 30720 < 32768)
NGROUPS = 33               # ceil(100000 / 3072) -> 33 groups
CHUNK = 32768              # table rows per group chunk
T = GROUP // N_CORES // P  # 3 nodes per partition per (group, core)
PER_GC = T * P             # 384 nodes per (group, core)
PAIRS_GC = PER_GC * S      # 3840 gathered rows per (group, core)
NODES_PADDED = NGROUPS * GROUP          # 101376
PER_CORE = NGROUPS * PER_GC             # 12672
TBL_ROWS = NGROUPS * CHUNK              # 1081344

_BUILT = None


def _build():
    from concourse import bacc, mybir
    import concourse.tile as tile

    nc = bacc.Bacc("TRN2", target_bir_lowering=False, debug=False,
                   num_swdge_queues=4)
    table = nc.dram_tensor(
        "table", (TBL_ROWS, D), mybir.dt.float32, kind="ExternalInput"
    )
    gidx = nc.dram_tensor(
        "gidx", (P, NGROUPS * PAIRS_GC // 16), mybir.dt.int16,
        kind="ExternalInput"
    )
    out = nc.dram_tensor(
        "out", (PER_CORE, D), mybir.dt.float32, kind="ExternalOutput"
    )

    W = PAIRS_GC // 16         # idx columns per group (240)
    GC = S * T                 # gather columns per group (30) of D floats
    HALF = PAIRS_GC // 2       # idxs per gather instruction (1920)
    HW_ = W // 2               # idx columns per half (120)
    HC = GC // 2               # gather columns per half (15)

    with tile.TileContext(nc) as tc:
        with tc.tile_pool(name="idxp", bufs=NGROUPS) as idxp, \
             tc.tile_pool(name="gp", bufs=6) as gp, \
             tc.tile_pool(name="rp", bufs=4) as rp:
            gidx_v = gidx.ap().rearrange("p (c w) -> p c w", c=NGROUPS)
            out_v = out.ap().rearrange("(c t p) d -> p c t d", p=P, t=T)
            gis = []
            for c in range(NGROUPS):
                gi = idxp.tile([P, W], mybir.dt.int16)
                nc.sync.dma_start(out=gi[:], in_=gidx_v[:, c])
                gis.append(gi)
            for c in range(NGROUPS):
                g = gp.tile([P, GC * D], mybir.dt.float32)
                for h in range(2):
                    nc.gpsimd.dma_gather(
                        out_ap=g[:, h * HC * D:(h + 1) * HC * D].rearrange(
                            "p (n d) -> p n d", d=D),
                        in_ap=table.ap()[c * CHUNK:(c + 1) * CHUNK, :],
                        idxs_ap=gis[c][:, h * HW_:(h + 1) * HW_],
                        num_idxs=HALF,
                        num_idxs_reg=HALF,
                        elem_size=D,
                        single_packet=False,
                        queue_num=(2 * c + h) % 4,
                    )
                # s-major tree-sum: columns are (s*T + t), t<T=3, s<10.
                # [s0..s4] += [s5..s9]; [s0 s1] += [s2 s3]; s0 += s1; s0 += s4
                TD = T * D
                nc.vector.tensor_add(
                    out=g[:, 0:5 * TD], in0=g[:, 0:5 * TD], in1=g[:, 5 * TD:10 * TD])
                nc.vector.tensor_add(
                    out=g[:, 0:2 * TD], in0=g[:, 0:2 * TD], in1=g[:, 2 * TD:4 * TD])
                nc.vector.tensor_add(
                    out=g[:, 0:TD], in0=g[:, 0:TD], in1=g[:, TD:2 * TD])
                red = rp.tile([P, TD], mybir.dt.float32)
                nc.vector.tensor_add(
                    out=red[:], in0=g[:, 0:TD], in1=g[:, 4 * TD:5 * TD])
                nc.sync.dma_start(out=out_v[:, c], in_=red[:])

    nc.compile()
    return nc


def _prepare(features_table, neigh_idx):
    """Host prep: relabeled 33-chunk table + per-core wrapped int16 idx
    streams + the output permutation. Returns (table_dev, gidx_per_core,
    out_perm) where out_perm[i] = original padded-node id of concat row i."""
    table_scaled = np.ascontiguousarray(
        np.asarray(features_table, dtype=np.float32) * np.float32(1.0 / S)
    )
    idx = np.asarray(neigh_idx).astype(np.int64)
    pad = NODES_PADDED - idx.shape[0]
    idx = np.concatenate([idx, np.zeros((pad, S), np.int64)], axis=0)

    # Relabel each group's referenced rows into a chunk, ordered by
    # (owning core, first occurrence in that core's gather stream): each
    # core's descriptors then walk ascending addresses in its own region
    # of the chunk (row-buffer-friendly HBM traffic). A row referenced by
    # several cores lives in the first core's region.
    table_dev = np.zeros((TBL_ROWS, D), dtype=np.float32)
    streams = np.empty((NGROUPS, N_CORES, PAIRS_GC), dtype=np.int64)
    for c in range(NGROUPS):
        block = idx[c * GROUP:(c + 1) * GROUP].reshape(N_CORES, T, P, S)
        # per-core gather stream order: position (s*T+t)*128+p
        streams[c] = block.transpose(0, 3, 1, 2).reshape(N_CORES, PAIRS_GC)

    gidx_cores_cols = [[] for _ in range(N_CORES)]
    for c in range(NGROUPS):
        flat = streams[c].ravel()                     # core-major stream concat
        u_first, first_pos = np.unique(flat, return_index=True)
        order = np.argsort(first_pos, kind="stable")  # first-occurrence order
        u = u_first[order]                            # chunk row -> orig row
        table_dev[c * CHUNK:c * CHUNK + len(u)] = table_scaled[u]
        remap = np.empty(len(u), dtype=np.int64)
        remap[order] = np.arange(len(u))              # unique-rank -> local id
        inv = remap[np.searchsorted(u_first, flat)]   # stream -> local id
        inv = inv.reshape(N_CORES, PAIRS_GC).astype(np.int16)
        for k in range(N_CORES):
            # wrap each 1920-idx half separately (one gather instruction each)
            halves = [
                np.tile(h.reshape(-1, 16).T, (8, 1))      # [128, 120]
                for h in inv[k].reshape(2, PAIRS_GC // 2)
            ]
            gidx_cores_cols[k].append(np.concatenate(halves, axis=1))
    gidx_cores = [np.ascontiguousarray(np.concatenate(cols, axis=1))
                  for cols in gidx_cores_cols]

    # concat row (k, c, t, p) -> padded node id
    k_, c_, t_, p_ = np.meshgrid(
        np.arange(N_CORES), np.arange(NGROUPS), np.arange(T), np.arange(P),
        indexing="ij")
    out_perm = (c_ * GROUP + k_ * PER_GC + t_ * P + p_).ravel()
    return table_dev, gidx_cores, out_perm


def kernel(features_table, neigh_idx):
    global _BUILT
    from concourse.bass_utils import run_bass_kernel_spmd

    table_dev, gidx_cores, out_perm = _prepare(features_table, neigh_idx)

    if _BUILT is None:
        _BUILT = _build()
    nc = _BUILT

    in_maps = [{"table": table_dev, "gidx": gidx_cores[k]}
               for k in range(N_CORES)]
    res = run_bass_kernel_spmd(nc, in_maps, core_ids=list(range(N_CORES)))
    rows = np.concatenate([r["out"] for r in res.results], axis=0)

    full = np.empty((NODES_PADDED, D), dtype=np.float32)
    full[out_perm] = rows
    return full[:BATCH]


# revision 7
# speedup vs baseline: 1.7825x; 1.7825x over previous
"""MeanAggregator (GNN mean message passing) Trainium2 kernel.

out[b, :] = mean_s features_table[neigh_idx[b, s], :]   b < 100000, s < 10

Strategy (data-parallel over the batch, 8 cores):

The SWDGE `dma_gather` primitive gathers thousands of 512B rows per
instruction but takes int16 indices (< 32768). Host-side we relabel:
nodes are split into 33 groups of 3072; each group references at most
3072*10 = 30720 unique table rows, so the host packs those rows into a
dedicated 32768-row chunk of a rebuilt table and rewrites the group's
indices as chunk-local int16. Every gather is then int16-addressable
AND in node order -- one gather per (group, core) lands the 10 neighbor
rows of 384 nodes s-major in SBUF, a 4-instruction DVE tree-sum
produces the 384 means, and a contiguous store writes them out. Rotating
the gathers across the 4 SWDGE queues lifts the DMA ceiling from
~52 GB/s (one queue's engines) to ~208 GB/s per core.

The table is pre-scaled by 1/10 on the host so the tree-sum directly
yields the mean; the per-core outputs are permuted back to the original
node order on the host.
"""

import numpy as np

P = 128            # SBUF partitions
D = 128            # feature dim
S = 10             # neighbors per node
N_NODES = 1_000_000
BATCH = 100_000
N_CORES = 8

GROUP = 3072               # nodes per group (unique refs <= 30720 < 32768)
NGROUPS = 33               # ceil(100000 / 3072) -> 33 groups
CHUNK = 32768              # table rows per group chunk
T = GROUP // N_CORES // P  # 3 nodes per partition per (group, core)
PER_GC = T * P             # 384 nodes per (group, core)
PAIRS_GC = PER_GC * S      # 3840 gathered rows per (group, core)
NODES_PADDED = NGROUPS * GROUP          # 101376
PER_CORE = NGROUPS * PER_GC             # 12672
TBL_ROWS = NGROUPS * CHUNK              # 1081344

_BUILT = None


def _build():
    from concourse import bacc, mybir
    import concourse.tile as tile

    nc = bacc.Bacc("TRN2", target_bir_lowering=False, debug=False,
                   num_swdge_queues=4)
    table = nc.dram_tensor(
        "table", (TBL_ROWS, D), mybir.dt.float32, kind="ExternalInput"
    )
    gidx = nc.dram_tensor(
        "gidx", (P, NGROUPS * PAIRS_GC // 16), mybir.dt.int16,
        kind="ExternalInput"
    )
    out = nc.dram_tensor(
        "out", (PER_CORE, D), mybir.dt.float32, kind="ExternalOutput"
    )

    W = PAIRS_GC // 16         # idx columns per group (240)
    GC = S * T                 # gather columns per group (30) of D floats
    HALF = PAIRS_GC // 2       # idxs per gather instruction (1920)
    HW_ = W // 2               # idx columns per half (120)
    HC = GC // 2               # gather columns per half (15)

    with tile.TileContext(nc) as tc:
        with tc.tile_pool(name="idxp", bufs=NGROUPS) as idxp, \
             tc.tile_pool(name="gp", bufs=8) as gp, \
             tc.tile_pool(name="rp", bufs=4) as rp:
            gidx_v = gidx.ap().rearrange("p (c w) -> p c w", c=NGROUPS)
            out_v = out.ap().rearrange("(c t p) d -> p c t d", p=P, t=T)
            gis = []
            for c in range(NGROUPS):
                gi = idxp.tile([P, W], mybir.dt.int16)
                nc.sync.dma_start(out=gi[:], in_=gidx_v[:, c])
                gis.append(gi)
            for c in range(NGROUPS):
                g = gp.tile([P, GC * D], mybir.dt.float32)
                for h in range(2):
                    nc.gpsimd.dma_gather(
                        out_ap=g[:, h * HC * D:(h + 1) * HC * D].rearrange(
                            "p (n d) -> p n d", d=D),
                        in_ap=table.ap()[c * CHUNK:(c + 1) * CHUNK, :],
                        idxs_ap=gis[c][:, h * HW_:(h + 1) * HW_],
                        num_idxs=HALF,
                        num_idxs_reg=HALF,
                        elem_size=D,
                        single_packet=False,
                        queue_num=(2 * c + h) % 4,
                    )
                # s-major tree-sum: columns are (s*T + t), t<T=3, s<10.
                # [s0..s4] += [s5..s9]; [s0 s1] += [s2 s3]; s0 += s1; s0 += s4
                TD = T * D
                nc.vector.tensor_add(
                    out=g[:, 0:5 * TD], in0=g[:, 0:5 * TD], in1=g[:, 5 * TD:10 * TD])
                nc.vector.tensor_add(
                    out=g[:, 0:2 * TD], in0=g[:, 0:2 * TD], in1=g[:, 2 * TD:4 * TD])
                nc.vector.tensor_add(
                    out=g[:, 0:TD], in0=g[:, 0:TD], in1=g[:, TD:2 * TD])
                red = rp.tile([P, TD], mybir.dt.float32)
                nc.vector.tensor_add(
                    out=red[:], in0=g[:, 0:TD], in1=g[:, 4 * TD:5 * TD])
                nc.sync.dma_start(out=out_v[:, c], in_=red[:])

    nc.compile()
    return nc


def _prepare(features_table, neigh_idx):
    """Host prep: relabeled 33-chunk table + per-core wrapped int16 idx
    streams + the output permutation. Returns (table_dev, gidx_per_core,
    out_perm) where out_perm[i] = original padded-node id of concat row i."""
    table_scaled = np.ascontiguousarray(
        np.asarray(features_table, dtype=np.float32) * np.float32(1.0 / S)
    )
    idx = np.asarray(neigh_idx).astype(np.int64)
    pad = NODES_PADDED - idx.shape[0]
    idx = np.concatenate([idx, np.zeros((pad, S), np.int64)], axis=0)

    # Relabel each group's referenced rows into a chunk, ordered by
    # (owning core, first occurrence in that core's gather stream): each
    # core's descriptors then walk ascending addresses in its own region
    # of the chunk (row-buffer-friendly HBM traffic). A row referenced by
    # several cores lives in the first core's region.
    table_dev = np.zeros((TBL_ROWS, D), dtype=np.float32)
    streams = np.empty((NGROUPS, N_CORES, PAIRS_GC), dtype=np.int64)
    for c in range(NGROUPS):
        block = idx[c * GROUP:(c + 1) * GROUP].reshape(N_CORES, T, P, S)
        # per-core gather stream order: position (s*T+t)*128+p
        streams[c] = block.transpose(0, 3, 1, 2).reshape(N_CORES, PAIRS_GC)

    gidx_cores_cols = [[] for _ in range(N_CORES)]
    for c in range(NGROUPS):
        flat = streams[c].ravel()                     # core-major stream concat
        u_first, first_pos = np.unique(flat, return_index=True)
        order = np.argsort(first_pos, kind="stable")  # first-occurrence order
        u = u_first[order]                            # chunk row -> orig row
        table_dev[c * CHUNK:c * CHUNK + len(u)] = table_scaled[u]
        remap = np.empty(len(u), dtype=np.int64)
        remap[order] = np.arange(len(u))              # unique-rank -> local id
        inv = remap[np.searchsorted(u_first, flat)]   # stream -> local id
        inv = inv.reshape(N_CORES, PAIRS_GC).astype(np.int16)
        for k in range(N_CORES):
            # wrap each 1920-idx half separately (one gather instruction each)
            halves = [
                np.tile(h.reshape(-1, 16).T, (8, 1))      # [128, 120]
                for h in inv[k].reshape(2, PAIRS_GC // 2)
            ]
            gidx_cores_cols[k].append(np.concatenate(halves, axis=1))
    gidx_cores = [np.ascontiguousarray(np.concatenate(cols, axis=1))
                  for cols in gidx_cores_cols]

    # concat row (k, c, t, p) -> padded node id
    k_, c_, t_, p_ = np.meshgrid(
        np.arange(N_CORES), np.arange(NGROUPS), np.arange(T), np.arange(P),
        indexing="ij")
    out_perm = (c_ * GROUP + k_ * PER_GC + t_ * P + p_).ravel()
    return table_dev, gidx_cores, out_perm


def kernel(features_table, neigh_idx):
    global _BUILT
    from concourse.bass_utils import run_bass_kernel_spmd

    table_dev, gidx_cores, out_perm = _prepare(features_table, neigh_idx)

    if _BUILT is None:
        _BUILT = _build()
    nc = _BUILT

    in_maps = [{"table": table_dev, "gidx": gidx_cores[k]}
               for k in range(N_CORES)]
    res = run_bass_kernel_spmd(nc, in_maps, core_ids=list(range(N_CORES)))
    rows = np.concatenate([r["out"] for r in res.results], axis=0)

    full = np.empty((NODES_PADDED, D), dtype=np.float32)
    full[out_perm] = rows
    return full[:BATCH]
